# revision 1
# baseline (speedup 1.0000x reference)
"""Trainium2 Bass kernel for nn_Attention_84585085927925 — Gram variant.

Reference (per batch element b, all fp32):
    qkv = x @ w_qkv.T ; q,k,v heads of 64 ; attn = sqrt(64) * q @ k.T (NO
    softmax) ; out = attn @ v ; out = out @ w_fc.T + b_fc

With no softmax the attention is associative, and k/v can be folded into
the weights via the Gram matrix of x:
    out_h = (s*q_h) @ (k_h.T @ v_h) = (s*q_h) @ (wk_h @ (x.T x) @ wv_h.T)
Per-core pipeline (one batch element per NeuronCore, 8 cores, no
collectives; e = output-feature axis, d = input-feature axis):
    qT   = (s*w_q).T-stationary @ xT       -> [768,1024]
    C    = x.T x  (x-stationary)           -> [768,768] (symmetric)
    T1   = C-stationary @ wv.T             -> [768,768]
    G    = wk.T-stationary @ T1 per pair   -> block-diag [128,128] per pair
    aoT  = G2-stationary @ qT per pair     -> [768,1024]
    outT = w_fc.T-stationary @ aoT + b_fc  -> [768,1024]
Host transposes x and outT. Matmuls run in float32r (~4x faster than
fp32, ~3e-4 relative error).
"""

import numpy as np

import concourse.bass as bass  # noqa: F401  (registers engine namespaces)
import concourse.mybir as mybir
import concourse.tile as tile
from concourse import bacc, bass_utils

F32 = mybir.dt.float32
F32R = mybir.dt.float32r

B, N, D, H = 8, 1024, 768, 12
HD = D // H            # 64
SCALE = float(np.sqrt(HD))
DT = D // 128          # 6  d-tiles
ET = D // 128          # 6  e-tiles
NT = N // 128          # 8  n(token)-tiles
NC2 = N // 512         # 2  512-wide token chunks
ECH = 384              # e-chunk that fits one PSUM bank with headroom
NPAIR = H // 2         # 6 head pairs


def _build_program():
    nc = bacc.Bacc(
        trn_type="TRN2", target_bir_lowering=False, debug=False, num_devices=B
    )
    xT_d = nc.dram_tensor("xT", [D, N], F32, kind="ExternalInput").ap()
    xN_d = nc.dram_tensor("xN", [N, D], F32, kind="ExternalInput").ap()
    wqT_d = nc.dram_tensor("wqT", [D, D], F32, kind="ExternalInput").ap()
    wkT_d = nc.dram_tensor("wkT", [D, D], F32, kind="ExternalInput").ap()
    wvT_d = nc.dram_tensor("wvT", [D, D], F32, kind="ExternalInput").ap()
    wfcT_d = nc.dram_tensor("wfcT", [D, D], F32, kind="ExternalInput").ap()
    bfc_d = nc.dram_tensor("bfc", [D], F32, kind="ExternalInput").ap()
    outT_d = nc.dram_tensor("outT", [D, N], F32, kind="ExternalOutput").ap()

    with tile.TileContext(nc) as tc:
        with tc.tile_pool(name="big", bufs=1) as big, \
             tc.tile_pool(name="wsp", bufs=3) as wsp, \
             tc.tile_pool(name="outsp", bufs=6) as outsp, \
             tc.tile_pool(name="ps", bufs=6, space="PSUM") as ps, \
             tc.tile_pool(name="psg", bufs=2, space="PSUM") as psg:

            xT_sb = big.tile([128, DT, N], F32R, name="xT_sb")
            xN_sb = big.tile([128, NT, D], F32R, name="xN_sb")
            qT_sb = big.tile([128, ET, N], F32R, name="qT_sb")
            c_sb = big.tile([128, DT, D], F32R, name="c_sb")
            t1_sb = big.tile([128, DT, D], F32R, name="t1_sb")
            ao_sb = big.tile([128, DT, N], F32R, name="ao_sb")
            g2_sb = big.tile([128, NPAIR, 128], F32R, name="g2_sb")
            bias_sb = big.tile([128, ET], F32, name="bias_sb")

            wq_r = wqT_d.rearrange("(o p) e -> p o e", p=128).bitcast(F32R)
            xT_r = xT_d.rearrange("(o p) n -> p o n", p=128).bitcast(F32R)
            xN_r = xN_d.rearrange("(o p) e -> p o e", p=128).bitcast(F32R)

            wq_tiles = []
            for et in range(ET):
                wq_t = wsp.tile([128, DT, 128], F32R, tag="w128", bufs=7,
                                name=f"wq_t{et}", uniquify=False)
                wq_tiles.append(wq_t)
            # first-needed data first: wq0 halves, xT n-half 0, then the rest
            for dh in range(2):
                dsl = slice(dh * 3, dh * 3 + 3)
                nc.sync.dma_start(wq_tiles[0][:, dsl, :], wq_r[:, dsl, 0:128])
            for dt in range(DT):
                nc.sync.dma_start(xT_sb[:, dt, 0:512], xT_r[:, dt, 0:512])
            for et in range(1, ET):
                for dh in range(2):
                    dsl = slice(dh * 3, dh * 3 + 3)
                    nc.sync.dma_start(wq_tiles[et][:, dsl, :],
                                      wq_r[:, dsl, et * 128:(et + 1) * 128])
            for dt in range(DT):
                nc.sync.dma_start(xT_sb[:, dt, 512:1024], xT_r[:, dt, 512:1024])
            for nt in range(NT):
                nc.sync.dma_start(xN_sb[:, nt, :], xN_r[:, nt, :])
            nc.sync.dma_start(bias_sb[:],
                              bfc_d.rearrange("(o p) -> p o", p=128))

            # ---- q.T projection: lhsT = wqT tile [d,e], rhs = xT chunk ----
            qt_chunks = [(0, 0, 256), (0, 256, 256),
                         (1, 0, 512), (2, 0, 512), (3, 0, 512),
                         (4, 0, 512), (5, 0, 512),
                         (0, 512, 512), (1, 512, 512), (2, 512, 512),
                         (3, 512, 512), (4, 512, 512), (5, 512, 512)]
            for et, off, width in qt_chunks:
                wq_t = wq_tiles[et]
                pt = ps.tile([128, 512], F32, tag="ps", name="pt_q")
                for dt in range(DT):
                    nc.tensor.matmul(
                        pt[:, :width],
                        wq_t[:, dt, :],
                        xT_sb[:, dt, off:off + width],
                        start=(dt == 0), stop=(dt == DT - 1),
                    )
                nc.vector.tensor_copy(
                    qT_sb[:, et, off:off + width], pt[:, :width]
                )

            # ---- C = x.T x : lhsT = x tile [n, d1], rhs = x [n, d2-chunk] --
            for ec in range(D // ECH):
                for d1t in range(DT):
                    pt = ps.tile([128, ECH], F32, tag="ps", name="pt_c")
                    for nt in range(NT):
                        nc.tensor.matmul(
                            pt[:],
                            xN_sb[:, nt, d1t * 128:(d1t + 1) * 128],
                            xN_sb[:, nt, ec * ECH:(ec + 1) * ECH],
                            start=(nt == 0), stop=(nt == NT - 1),
                        )
                    nc.vector.tensor_copy(
                        c_sb[:, d1t, ec * ECH:(ec + 1) * ECH], pt[:]
                    )

            # ---- T1 = C @ wv.T : lhsT = C tile (symmetric), rhs = wvT ----
            wv_r = wvT_d.rearrange("(o p) e -> p o e", p=128).bitcast(F32R)
            for ec in range(D // ECH):
                wv_t = wsp.tile([128, DT, ECH], F32R, tag="w384",
                                name=f"wv_t{ec}", uniquify=False)
                for dh in range(3):
                    dsl = slice(dh * 2, dh * 2 + 2)
                    nc.sync.dma_start(
                        wv_t[:, dsl, :],
                        wv_r[:, dsl, ec * ECH:(ec + 1) * ECH],
                    )
                for d1t in range(DT):
                    pt = ps.tile([128, ECH], F32, tag="ps", name="pt_t1")
                    for d2t in range(DT):
                        nc.tensor.matmul(
                            pt[:],
                            c_sb[:, d2t, d1t * 128:(d1t + 1) * 128],
                            wv_t[:, d2t, :],
                            start=(d2t == 0), stop=(d2t == DT - 1),
                        )
                    nc.vector.tensor_copy(
                        t1_sb[:, d1t, ec * ECH:(ec + 1) * ECH], pt[:]
                    )

            # ---- G = wk @ T1 per head pair, stored block-diagonal ----
            wk_r = wkT_d.rearrange("(o p) e -> p o e", p=128).bitcast(F32R)
            for t in range(NPAIR):
                wk_t = wsp.tile([128, DT, 128], F32R, tag="w128", bufs=7,
                                name=f"wk_t{t}", uniquify=False)
                nc.sync.dma_start(wk_t[:], wk_r[:, :, t * 128:(t + 1) * 128])
                gp = psg.tile([128, 128], F32, tag="psg", name="gp")
                for dt in range(DT):
                    nc.tensor.matmul(
                        gp[:],
                        wk_t[:, dt, :],
                        t1_sb[:, dt, t * 128:(t + 1) * 128],
                        start=(dt == 0), stop=(dt == DT - 1),
                    )
                nc.vector.tensor_scalar_mul(g2_sb[:, t, :], gp[:], 0.0)
                nc.vector.tensor_copy(g2_sb[0:64, t, 0:64], gp[0:64, 0:64])
                nc.vector.tensor_copy(g2_sb[64:128, t, 64:128],
                                      gp[64:128, 64:128])

            # ---- attn-out.T then fc, interleaved per 512-chunk ----
            wfc_r = wfcT_d.rearrange("(o p) e -> p o e", p=128).bitcast(F32R)
            wfc_tiles = []
            for et in range(ET):
                wfc_t = wsp.tile([128, DT, 128], F32R, tag="w128", bufs=7,
                                 name=f"wfc_t{et}", uniquify=False)
                wfc_tiles.append(wfc_t)
                nc.sync.dma_start(wfc_t[:], wfc_r[:, :, et * 128:(et + 1) * 128])
            for ic in range(NC2):
                for t in range(NPAIR):
                    pt = ps.tile([128, 512], F32, tag="ps", name="pt_ao")
                    nc.tensor.matmul(
                        pt[:],
                        g2_sb[:, t, :],
                        qT_sb[:, t, ic * 512:(ic + 1) * 512],
                        start=True, stop=True,
                    )
                    dst_ap = ao_sb[:, t, ic * 512:(ic + 1) * 512]
                    if t % 2 == 0:
                        nc.vector.tensor_copy(dst_ap, pt[:])
                    else:
                        nc.scalar.copy(dst_ap, pt[:])
                for et in range(ET):
                    wfc_t = wfc_tiles[et]
                    pt = ps.tile([128, 512], F32, tag="ps", name="pt_fc")
                    for dt in range(DT):
                        nc.tensor.matmul(
                            pt[:],
                            wfc_t[:, dt, :],
                            ao_sb[:, dt, ic * 512:(ic + 1) * 512],
                            start=(dt == 0), stop=(dt == DT - 1),
                        )
                    ot = outsp.tile([128, 512], F32, tag="ot", name="ot")
                    nc.scalar.add(ot[:], pt[:], bias_sb[:, et:et + 1])
                    last = (ic == NC2 - 1 and et == ET - 1)
                    nsplit = 4 if last else 1
                    w = 128 // nsplit
                    for ph in range(nsplit):
                        nc.sync.dma_start(
                            outT_d[et * 128 + ph * w:et * 128 + (ph + 1) * w,
                                   ic * 512:(ic + 1) * 512],
                            ot[ph * w:(ph + 1) * w, :],
                        )

    nc.compile()
    return nc


_NC_CACHE = None
LAST_EXEC_NS = None
LAST_RES = None


def kernel(x, w_qkv, w_fc, b_fc, _trace=False):
    global _NC_CACHE, LAST_EXEC_NS, LAST_RES
    x = np.asarray(x, dtype=np.float32)
    w_qkv = np.asarray(w_qkv, dtype=np.float32)
    w_fc = np.asarray(w_fc, dtype=np.float32)
    b_fc = np.asarray(b_fc, dtype=np.float32)

    if _NC_CACHE is None:
        _NC_CACHE = _build_program()
    nc = _NC_CACHE

    wqT = np.ascontiguousarray((SCALE * w_qkv[:D]).T)
    wkT = np.ascontiguousarray(w_qkv[D:2 * D].T)
    wvT = np.ascontiguousarray(w_qkv[2 * D:].T)
    wfcT = np.ascontiguousarray(w_fc.T)

    in_maps = []
    for b in range(B):
        in_maps.append({
            "xT": np.ascontiguousarray(x[b].T),
            "xN": np.ascontiguousarray(x[b]),
            "wqT": wqT, "wkT": wkT, "wvT": wvT, "wfcT": wfcT,
            "bfc": b_fc,
        })

    res = bass_utils.run_bass_kernel_spmd(
        nc, in_maps, core_ids=list(range(B)), trace=_trace
    )
    LAST_EXEC_NS = res.exec_time_ns
    LAST_RES = res
    out = np.stack([res.results[b]["outT"].T for b in range(B)])
    return np.ascontiguousarray(out.astype(np.float32))



# revision 2
# speedup vs baseline: 1.0571x; 1.0571x over previous
"""Trainium2 Bass kernel for nn_Attention_84585085927925 — folded Gram chain.

Reference (per batch element b, fp32):
    qkv = x @ w_qkv.T ; q,k,v heads of 64 ; attn = sqrt(64) * q @ k.T (NO
    softmax) ; out = attn @ v ; out = out @ w_fc.T + b_fc

With no softmax the whole per-batch map is linear in x on the right and
collapses into a single batch-dependent weight matrix:
    G_h  = wk_h (x^T x) wv_h^T                (64 x 64 per head)
    W    = sum_h (s*wq_h)^T G_h wfc_h^T       (768 x 768)
    out  = x @ W + b_fc
Per-core pipeline (one batch element per core, 8 cores, bf16 matmuls):
    C    = x^T x                                36,864 PE cycles
    M1T  = C @ wv^T      (= (wv C)^T)           27,648
    G'_t = M1T_pair^T wk_pair^T  (diag 64-blks)  4,608
    GF   = blockdiag(G'_t) @ wfc^T               4,608
    W    = (s*wq)^T @ GF                        27,648
    outT = W^T-tiles @ x^T                      36,864
All HBM tensors are host-prearranged to [128, ...] partition-major so every
DMA is 128 contiguous descriptors.  bf16 keeps matmuls at 1 cycle/column
regardless of width (rel err ~5e-3, threshold 2e-2).
"""

import numpy as np
import ml_dtypes

import concourse.bass as bass  # noqa: F401  (registers engine namespaces)
import concourse.mybir as mybir
import concourse.tile as tile
from concourse import bacc, bass_utils

F32 = mybir.dt.float32
BF16 = mybir.dt.bfloat16

B, N, D, H = 8, 1024, 768, 12
HD = D // H            # 64
SCALE = float(np.sqrt(HD))
DT = D // 128          # 6 chunks of the feature axis
NT = N // 128          # 8 token tiles
NPAIR = H // 2         # 6 head pairs
ECH = 384              # column chunk (fits PSUM bank with headroom)


def _build_program():
    nc = bacc.Bacc(
        trn_type="TRN2", target_bir_lowering=False, debug=False, num_devices=B
    )
    xn_d = nc.dram_tensor("xn", [128, NT, D], BF16, kind="ExternalInput").ap()
    xt_d = nc.dram_tensor("xt", [128, DT, N], BF16, kind="ExternalInput").ap()
    wvt_d = nc.dram_tensor("wvt", [128, DT, D], BF16, kind="ExternalInput").ap()
    wkt_d = nc.dram_tensor("wkt", [128, DT, D], BF16, kind="ExternalInput").ap()
    wq_d = nc.dram_tensor("wq", [128, DT, D], BF16, kind="ExternalInput").ap()
    wfct_d = nc.dram_tensor("wfct", [128, DT, D], BF16,
                            kind="ExternalInput").ap()
    bias_d = nc.dram_tensor("bias", [128, DT], F32, kind="ExternalInput").ap()
    out_d = nc.dram_tensor("outT", [128, 12, 512], BF16,
                           kind="ExternalOutput").ap()

    with tile.TileContext(nc) as tc:
        with tc.tile_pool(name="big", bufs=1) as big, \
             tc.tile_pool(name="outsp", bufs=4) as outsp, \
             tc.tile_pool(name="ps", bufs=7, space="PSUM") as ps:

            xn_sb = big.tile([128, NT, D], BF16, name="xn_sb")
            xt_sb = big.tile([128, DT, N], BF16, name="xt_sb")
            wvt_sb = big.tile([128, DT, D], BF16, name="wvt_sb")
            wkt_sb = big.tile([128, DT, D], BF16, name="wkt_sb")
            wq_sb = big.tile([128, DT, D], BF16, name="wq_sb")
            wfct_sb = big.tile([128, DT, D], BF16, name="wfct_sb")
            c_sb = big.tile([128, DT, D], BF16, name="c_sb")
            m1t_sb = big.tile([128, DT, D], BF16, name="m1t_sb")
            g2_sb = big.tile([128, NPAIR, 128], BF16, name="g2_sb")
            gf_sb = big.tile([128, DT, D], BF16, name="gf_sb")
            w_sb = big.tile([128, DT, D], BF16, name="w_sb")
            bias_sb = big.tile([128, DT], F32, name="bias_sb")
            warm_sb = big.tile([128, 512], BF16, name="warm_sb")

            # ---- DMA schedule: x first (C needs it), weights behind ----
            nc.vector.memset(warm_sb[:], 0.0)
            nc.vector.memset(g2_sb[:], 0.0)
            for ch in range(4):
                nc.sync.dma_start(xn_sb[:, 2 * ch:2 * ch + 2, :],
                                  xn_d[:, 2 * ch:2 * ch + 2, :])
            nc.sync.dma_start(wvt_sb[:], wvt_d[:])
            nc.sync.dma_start(wkt_sb[:], wkt_d[:])
            nc.sync.dma_start(wfct_sb[:], wfct_d[:])
            nc.sync.dma_start(wq_sb[:], wq_d[:])
            nc.sync.dma_start(xt_sb[:], xt_d[:])
            nc.sync.dma_start(bias_sb[:], bias_d[:])

            # ---- PE warmup while the first x chunk streams in ----
            wpt = ps.tile([128, 512], F32, tag="b", name="wpt")
            for i in range(4):
                nc.tensor.matmul(wpt[:], warm_sb[:, 0:128], warm_sb[:],
                                 start=(i == 0), stop=(i == 3))

            # ---- C = x^T x, col-halves as passes, streaming over n ----
            for jc in range(2):
                cacc = [ps.tile([128, ECH], F32, tag="b", name=f"cacc{jc}{d}")
                        for d in range(DT)]
                for nt in range(NT):
                    lhs_nt = xn_sb[:, nt, :]
                    for d1 in range(DT):
                        nc.tensor.matmul(
                            cacc[d1][:],
                            lhs_nt[:, d1 * 128:(d1 + 1) * 128],
                            lhs_nt[:, jc * ECH:(jc + 1) * ECH],
                            start=(nt == 0), stop=(nt == NT - 1),
                        )
                for d1 in range(DT):
                    eng = nc.vector if d1 % 2 == 0 else nc.scalar
                    dst = c_sb[:, d1, jc * ECH:(jc + 1) * ECH]
                    if d1 % 2 == 0:
                        eng.tensor_copy(dst, cacc[d1][:])
                    else:
                        eng.copy(dst, cacc[d1][:])

            # ---- M1T = C @ wv^T : lhsT = C block (k,d), rhs = wvT ----
            for jc in range(2):
                for d in range(DT):
                    pt = ps.tile([128, ECH], F32, tag="b", name="pt_m1")
                    for k in range(DT):
                        nc.tensor.matmul(
                            pt[:],
                            c_sb[:, k, d * 128:(d + 1) * 128],
                            wvt_sb[:, k, jc * ECH:(jc + 1) * ECH],
                            start=(k == 0), stop=(k == DT - 1),
                        )
                    dst = m1t_sb[:, d, jc * ECH:(jc + 1) * ECH]
                    if d % 2 == 0:
                        nc.vector.tensor_copy(dst, pt[:])
                    else:
                        nc.scalar.copy(dst, pt[:])

            # ---- G' per head pair: diag 64-blocks of M1T_p^T @ wkT_p ----
            for t in range(NPAIR):
                gp = ps.tile([128, 128], F32, tag="b", name="gp")
                for k in range(DT):
                    nc.tensor.matmul(
                        gp[:],
                        m1t_sb[:, k, t * 128:(t + 1) * 128],
                        wkt_sb[:, k, t * 128:(t + 1) * 128],
                        start=(k == 0), stop=(k == DT - 1),
                    )
                nc.vector.tensor_copy(g2_sb[0:64, t, 0:64], gp[0:64, 0:64])
                nc.vector.tensor_copy(g2_sb[64:128, t, 64:128],
                                      gp[64:128, 64:128])

            # ---- GF = blockdiag(G') @ wfc^T ----
            for jc in range(2):
                for t in range(NPAIR):
                    pt = ps.tile([128, ECH], F32, tag="b", name="pt_gf")
                    nc.tensor.matmul(
                        pt[:],
                        g2_sb[:, t, :],
                        wfct_sb[:, t, jc * ECH:(jc + 1) * ECH],
                        start=True, stop=True,
                    )
                    dst = gf_sb[:, t, jc * ECH:(jc + 1) * ECH]
                    if t % 2 == 0:
                        nc.vector.tensor_copy(dst, pt[:])
                    else:
                        nc.scalar.copy(dst, pt[:])

            # ---- W = (s*wq)^T @ GF ----
            for jc in range(2):
                for d in range(DT):
                    pt = ps.tile([128, ECH], F32, tag="b", name="pt_w")
                    for k in range(DT):
                        nc.tensor.matmul(
                            pt[:],
                            wq_sb[:, k, d * 128:(d + 1) * 128],
                            gf_sb[:, k, jc * ECH:(jc + 1) * ECH],
                            start=(k == 0), stop=(k == DT - 1),
                        )
                    dst = w_sb[:, d, jc * ECH:(jc + 1) * ECH]
                    if d % 2 == 0:
                        nc.vector.tensor_copy(dst, pt[:])
                    else:
                        nc.scalar.copy(dst, pt[:])

            # ---- outT[j, n] = sum_d W[d, j] x^T[d, n], + bias, DMA out ----
            for jt in range(DT):
                for ic in range(2):
                    pt = ps.tile([128, 512], F32, tag="b", name="pt_o")
                    for k in range(DT):
                        nc.tensor.matmul(
                            pt[:],
                            w_sb[:, k, jt * 128:(jt + 1) * 128],
                            xt_sb[:, k, ic * 512:(ic + 1) * 512],
                            start=(k == 0), stop=(k == DT - 1),
                        )
                    ot = outsp.tile([128, 512], BF16, tag="ot", name="ot")
                    nc.scalar.add(ot[:], pt[:], bias_sb[:, jt:jt + 1])
                    last = (jt == DT - 1 and ic == 1)
                    nsplit = 4 if last else 1
                    w = 128 // nsplit
                    for ph in range(nsplit):
                        nc.sync.dma_start(
                            out_d[ph * w:(ph + 1) * w, jt * 2 + ic, :],
                            ot[ph * w:(ph + 1) * w, :],
                        )

    nc.compile()
    return nc


_NC_CACHE = None
LAST_EXEC_NS = None
LAST_RES = None


def _arr128(a):
    """[D0*128, M] row-major -> [128, D0, M] partition-major, contiguous."""
    d0 = a.shape[0] // 128
    return np.ascontiguousarray(
        a.reshape(d0, 128, a.shape[1]).transpose(1, 0, 2))


def kernel(x, w_qkv, w_fc, b_fc, _trace=False):
    global _NC_CACHE, LAST_EXEC_NS, LAST_RES
    x = np.asarray(x, dtype=np.float32)
    w_qkv = np.asarray(w_qkv, dtype=np.float32)
    w_fc = np.asarray(w_fc, dtype=np.float32)
    b_fc = np.asarray(b_fc, dtype=np.float32)

    if _NC_CACHE is None:
        _NC_CACHE = _build_program()
    nc = _NC_CACHE

    bf = ml_dtypes.bfloat16
    wq = _arr128((SCALE * w_qkv[:D]).astype(bf))          # [128, 6, 768] e-major
    wkt = _arr128(np.ascontiguousarray(w_qkv[D:2 * D].T).astype(bf))
    wvt = _arr128(np.ascontiguousarray(w_qkv[2 * D:].T).astype(bf))
    wfct = _arr128(np.ascontiguousarray(w_fc.T).astype(bf))
    bias = np.ascontiguousarray(b_fc.reshape(DT, 128).T)  # [128, 6] f32

    in_maps = []
    for b in range(B):
        xb = x[b].astype(bf)                              # [1024, 768]
        in_maps.append({
            "xn": _arr128(xb),                            # [128, 8, 768]
            "xt": _arr128(np.ascontiguousarray(xb.T)),    # [128, 6, 1024]
            "wvt": wvt, "wkt": wkt, "wq": wq, "wfct": wfct,
            "bias": bias,
        })

    res = bass_utils.run_bass_kernel_spmd(
        nc, in_maps, core_ids=list(range(B)), trace=_trace
    )
    LAST_EXEC_NS = res.exec_time_ns
    LAST_RES = res
    outs = []
    for b in range(B):
        a = res.results[b]["outT"]                        # [128, 12, 512] bf16
        a = a.reshape(128, DT, 2, 512).transpose(1, 0, 2, 3).reshape(D, N)
        outs.append(a.T.astype(np.float32))
    return np.ascontiguousarray(np.stack(outs))


# revision 13
# speedup vs baseline: 1.2218x; 1.1558x over previous
"""Trainium2 Bass kernel for nn_Attention_84585085927925 — folded Gram chain.

Reference (per batch element b, fp32):
    qkv = x @ w_qkv.T ; q,k,v heads of 64 ; attn = sqrt(64) * q @ k.T (NO
    softmax) ; out = attn @ v ; out = out @ w_fc.T + b_fc

No softmax -> the map is linear and collapses into a per-batch weight:
    G_h  = wk_h (x^T x) wv_h^T                (64 x 64 per head)
    W    = sum_h (s*wq_h)^T G_h wfc_h^T       (768 x 768)
    out  = x @ W + b_fc
Per-core pipeline (one batch element per core, 8 cores):
    C    = x^T x        symmetric: 21 upper blocks (bf16) + 15 PE transposes
    M1T  = C @ wv^T     (f32r, = (wv C)^T)
    G'_t = diag 64-blocks of M1T_pair^T wk_pair  (bf16, narrow)
    GF   = blockdiag(G') @ wfc^T                 (f32r)
    W    = (s*wq)^T @ GF                         (f32r)
    outT = W^T-tiles @ x^T + bias                (f32r)
Measured rates: f32r matmul ~0.44 ns/col (>=256 wide), bf16 ~0.505 ns/col
(any width).  f32r for wide phases, bf16 for C (cheap x streaming, narrow
C blocks) and G'.  HBM tensors are host-prearranged to [128, ...] so every
DMA is 128 contiguous descriptors; DMA triggers cost ~700ns serialized on
their queue engine, so output triggers alternate sync/gpsimd.
"""

import numpy as np
import ml_dtypes

import concourse.bass as bass  # noqa: F401  (registers engine namespaces)
import concourse.mybir as mybir
import concourse.tile as tile
from concourse import bacc, bass_utils

F32 = mybir.dt.float32
F32R = mybir.dt.float32r
BF16 = mybir.dt.bfloat16

B, N, D, H = 8, 1024, 768, 12
HD = D // H            # 64
SCALE = float(np.sqrt(HD))
DT = D // 128          # 6 chunks of the feature axis
NT = N // 128          # 8 token tiles
NPAIR = H // 2         # 6 head pairs
ECH = 384              # column chunk (fits PSUM bank with headroom)


def _build_program():
    nc = bacc.Bacc(
        trn_type="TRN2", target_bir_lowering=False, debug=False, num_devices=B
    )
    xn_d = nc.dram_tensor("xn", [128, NT, D], BF16, kind="ExternalInput").ap()
    xt_d = nc.dram_tensor("xt", [128, DT, N], F32, kind="ExternalInput").ap()
    wvt_d = nc.dram_tensor("wvt", [128, DT, D], F32, kind="ExternalInput").ap()
    wkt_d = nc.dram_tensor("wkt", [128, DT, D], BF16,
                           kind="ExternalInput").ap()
    wq_d = nc.dram_tensor("wq", [128, DT, D], F32, kind="ExternalInput").ap()
    wfct_d = nc.dram_tensor("wfct", [128, DT, D], F32,
                            kind="ExternalInput").ap()
    idm_d = nc.dram_tensor("idm", [128, 128], BF16, kind="ExternalInput").ap()
    bias_d = nc.dram_tensor("bias", [128, DT], F32, kind="ExternalInput").ap()
    out_d = nc.dram_tensor("outT", [128, 12, 512], BF16,
                           kind="ExternalOutput").ap()

    with tile.TileContext(nc) as tc:
        with tc.tile_pool(name="big", bufs=1) as big, \
             tc.tile_pool(name="tmps", bufs=10) as tmps, \
             tc.tile_pool(name="outsp", bufs=4) as outsp, \
             tc.tile_pool(name="ps", bufs=7, space="PSUM") as ps:

            xn_sb = big.tile([128, NT, D], BF16, name="xn_sb")
            xt_sb = big.tile([128, DT, N], F32R, name="xt_sb")
            wvt_sb = big.tile([128, DT, D], F32R, name="wvt_sb")
            wkt_sb = big.tile([128, DT, D], BF16, name="wkt_sb")
            wq_sb = big.tile([128, DT, D], F32R, name="wq_sb")
            wfct_sb = big.tile([128, DT, D], F32R, name="wfct_sb")
            c_sb = big.tile([128, DT, D], F32R, name="c_sb")
            m1t_sb = big.tile([128, DT, D], BF16, name="m1t_sb")
            g2_sb = big.tile([128, NPAIR, 128], F32R, name="g2_sb")
            gf_sb = big.tile([128, DT, D], F32R, name="gf_sb")
            w_sb = big.tile([128, DT, D], F32R, name="w_sb")
            id_sb = big.tile([128, 128], BF16, name="id_sb")
            bias_sb = big.tile([128, DT], F32, name="bias_sb")
            warm_sb = big.tile([128, 512], BF16, name="warm_sb")

            xtr = xt_d.bitcast(F32R)
            wvtr = wvt_d.bitcast(F32R)
            wqr = wq_d.bitcast(F32R)
            wfctr = wfct_d.bitcast(F32R)

            # early memset off the critical engines
            nc.gpsimd.memset(warm_sb[:], 0.0)

            # ---- DMA triggers, priority order (x first for C) ----
            nc.sync.dma_start(xn_sb[:, 0:1, :], xn_d[:, 0:1, :])
            nc.sync.dma_start(xn_sb[:, 1:2, :], xn_d[:, 1:2, :])
            nc.sync.dma_start(xn_sb[:, 2:4, :], xn_d[:, 2:4, :])
            nc.sync.dma_start(xn_sb[:, 4:6, :], xn_d[:, 4:6, :])
            nc.sync.dma_start(xn_sb[:, 6:8, :], xn_d[:, 6:8, :])
            nc.sync.dma_start(id_sb[:], idm_d[:])
            nc.sync.dma_start(wvt_sb[:], wvtr[:])
            nc.sync.dma_start(wkt_sb[:], wkt_d[:])
            nc.sync.dma_start(wfct_sb[:], wfctr[:])
            nc.sync.dma_start(wq_sb[:], wqr[:])
            nc.sync.dma_start(xt_sb[:], xtr[:])
            nc.sync.dma_start(bias_sb[:], bias_d[:])

            # ---- PE warmup (p-state ramp) while x streams in ----
            wpt = ps.tile([128, 512], F32, tag="b", name="wpt")
            for i in range(3):
                nc.tensor.matmul(wpt[:], warm_sb[:, 0:128], warm_sb[:],
                                 start=(i == 0), stop=(i == 2))

            # ---- C = x^T x, upper triangle by diagonal bands (bf16) ----
            # band k holds blocks (d, d+k); lower blocks come from PE
            # transposes scheduled two bands later.
            tmt = {}   # (d1, d2) -> bf16 staging tile for transpose

            def c_band(k):
                accs = []
                for d in range(DT - k):
                    acc = ps.tile([128, 128], F32, tag="b", name=f"cb{k}{d}")
                    accs.append(acc)
                for nt in range(NT):
                    for d in range(DT - k):
                        nc.tensor.matmul(
                            accs[d][:],
                            xn_sb[:, nt, d * 128:(d + 1) * 128],
                            xn_sb[:, nt, (d + k) * 128:(d + k + 1) * 128],
                            start=(nt == 0), stop=(nt == NT - 1),
                        )
                for d in range(DT - k):
                    dst = c_sb[:, d, (d + k) * 128:(d + k + 1) * 128]
                    if d % 2 == 0:
                        nc.vector.tensor_copy(dst, accs[d][:])
                    else:
                        nc.scalar.copy(dst, accs[d][:])
                    if k > 0:
                        tm = tmps.tile([128, 128], BF16, tag="tm",
                                       name=f"tm{k}{d}")
                        nc.vector.tensor_copy(tm[:], accs[d][:])
                        tmt[(d, d + k)] = tm

            def c_transpose(k):
                # emit transposes for band k: block (d, d+k) -> (d+k, d)
                for d in range(DT - k):
                    tm = tmt.pop((d, d + k))
                    pt = ps.tile([128, 128], BF16, tag="b", name=f"tp{k}{d}")
                    nc.tensor.transpose(pt[:], tm[:], id_sb[:])
                    dst = c_sb[:, d + k, d * 128:(d + 1) * 128]
                    if d % 2 == 0:
                        nc.scalar.copy(dst, pt[:])
                    else:
                        nc.vector.tensor_copy(dst, pt[:])

            c_band(0)
            c_band(1)
            c_band(2)
            c_transpose(1)
            c_band(3)
            c_transpose(2)
            c_band(4)
            c_transpose(3)
            c_band(5)
            c_transpose(4)
            c_transpose(5)

            # ---- M1T = C @ wv^T (f32r); d descending so the late
            # transposes (needed by small d) have maximal slack ----
            for d in range(DT - 1, -1, -1):
                for jc in range(2):
                    pt = ps.tile([128, ECH], F32, tag="b", name="pt_m1")
                    for k in range(DT):
                        nc.tensor.matmul(
                            pt[:],
                            c_sb[:, k, d * 128:(d + 1) * 128],
                            wvt_sb[:, k, jc * ECH:(jc + 1) * ECH],
                            start=(k == 0), stop=(k == DT - 1),
                        )
                    dst = m1t_sb[:, d, jc * ECH:(jc + 1) * ECH]
                    if jc == 0:
                        nc.vector.tensor_copy(dst, pt[:])
                    else:
                        nc.scalar.copy(dst, pt[:])

            # ---- G' per head pair: diag 64-blocks of M1T_p^T @ wkT_p ----
            for t in range(NPAIR):
                gp = ps.tile([128, 128], F32, tag="b", name="gp")
                for k in range(DT):
                    nc.tensor.matmul(
                        gp[:],
                        m1t_sb[:, k, t * 128:(t + 1) * 128],
                        wkt_sb[:, k, t * 128:(t + 1) * 128],
                        start=(k == 0), stop=(k == DT - 1),
                    )
                nc.vector.tensor_scalar_mul(g2_sb[:, t, :], gp[:], 0.0)
                nc.vector.tensor_copy(g2_sb[0:64, t, 0:64], gp[0:64, 0:64])
                nc.vector.tensor_copy(g2_sb[64:128, t, 64:128],
                                      gp[64:128, 64:128])

            # ---- GF = blockdiag(G') @ wfc^T (f32r) ----
            for jc in range(2):
                for t in range(NPAIR):
                    pt = ps.tile([128, ECH], F32, tag="b", name="pt_gf")
                    nc.tensor.matmul(
                        pt[:],
                        g2_sb[:, t, :],
                        wfct_sb[:, t, jc * ECH:(jc + 1) * ECH],
                        start=True, stop=True,
                    )
                    dst = gf_sb[:, t, jc * ECH:(jc + 1) * ECH]
                    if t % 2 == 0:
                        nc.vector.tensor_copy(dst, pt[:])
                    else:
                        nc.scalar.copy(dst, pt[:])

            # ---- W = (s*wq)^T @ GF (f32r) ----
            for jc in range(2):
                for d in range(DT):
                    pt = ps.tile([128, ECH], F32, tag="b", name="pt_w")
                    for k in range(DT):
                        nc.tensor.matmul(
                            pt[:],
                            wq_sb[:, k, d * 128:(d + 1) * 128],
                            gf_sb[:, k, jc * ECH:(jc + 1) * ECH],
                            start=(k == 0), stop=(k == DT - 1),
                        )
                    dst = w_sb[:, d, jc * ECH:(jc + 1) * ECH]
                    if d % 2 == 0:
                        nc.vector.tensor_copy(dst, pt[:])
                    else:
                        nc.scalar.copy(dst, pt[:])

            # ---- outT[j, n] = sum_d W[d, j] x^T[d, n] + bias ----
            trig = [nc.sync, nc.gpsimd]
            for jt in range(DT):
                for ic in range(2):
                    pt = ps.tile([128, 512], F32, tag="b", name="pt_o")
                    for k in range(DT):
                        nc.tensor.matmul(
                            pt[:],
                            w_sb[:, k, jt * 128:(jt + 1) * 128],
                            xt_sb[:, k, ic * 512:(ic + 1) * 512],
                            start=(k == 0), stop=(k == DT - 1),
                        )
                    ot = outsp.tile([128, 512], BF16, tag="ot", name="ot")
                    last = (jt == DT - 1 and ic == 1)
                    if not last:
                        nc.scalar.add(ot[:], pt[:], bias_sb[:, jt:jt + 1])
                        trig[(jt * 2 + ic) % 2].dma_start(
                            out_d[:, jt * 2 + ic, :], ot[:])
                    else:
                        # split the tail: two engines add, two engines DMA
                        nc.scalar.add(ot[:, 0:256], pt[:, 0:256],
                                      bias_sb[:, jt:jt + 1])
                        nc.vector.tensor_scalar_add(
                            ot[:, 256:512], pt[:, 256:512],
                            bias_sb[:, jt:jt + 1])
                        nc.sync.dma_start(out_d[:, jt * 2 + ic, 0:256],
                                          ot[:, 0:256])
                        nc.gpsimd.dma_start(out_d[:, jt * 2 + ic, 256:512],
                                            ot[:, 256:512])

    nc.compile()
    return nc


_NC_CACHE = None
LAST_EXEC_NS = None
LAST_RES = None


def _arr128(a):
    """[D0*128, M] row-major -> [128, D0, M] partition-major, contiguous."""
    d0 = a.shape[0] // 128
    return np.ascontiguousarray(
        a.reshape(d0, 128, a.shape[1]).transpose(1, 0, 2))


def kernel(x, w_qkv, w_fc, b_fc, _trace=False):
    global _NC_CACHE, LAST_EXEC_NS, LAST_RES
    x = np.asarray(x, dtype=np.float32)
    w_qkv = np.asarray(w_qkv, dtype=np.float32)
    w_fc = np.asarray(w_fc, dtype=np.float32)
    b_fc = np.asarray(b_fc, dtype=np.float32)

    if _NC_CACHE is None:
        _NC_CACHE = _build_program()
    nc = _NC_CACHE

    bf = ml_dtypes.bfloat16
    wq = _arr128(SCALE * w_qkv[:D])                       # [128, 6, 768] f32
    wkt = _arr128(np.ascontiguousarray(w_qkv[D:2 * D].T).astype(bf))
    wvt = _arr128(np.ascontiguousarray(w_qkv[2 * D:].T))
    wfct = _arr128(np.ascontiguousarray(w_fc.T))
    bias = np.ascontiguousarray(b_fc.reshape(DT, 128).T)  # [128, 6] f32
    idm = np.eye(128, dtype=bf)

    in_maps = []
    for b in range(B):
        in_maps.append({
            "xn": _arr128(x[b].astype(bf)),               # [128, 8, 768]
            "xt": _arr128(np.ascontiguousarray(x[b].T)),  # [128, 6, 1024] f32
            "wvt": wvt, "wkt": wkt, "wq": wq, "wfct": wfct,
            "idm": idm, "bias": bias,
        })

    res = bass_utils.run_bass_kernel_spmd(
        nc, in_maps, core_ids=list(range(B)), trace=_trace
    )
    LAST_EXEC_NS = res.exec_time_ns
    LAST_RES = res
    outs = []
    for b in range(B):
        a = res.results[b]["outT"]                        # [128, 12, 512] bf16
        a = a.reshape(128, DT, 2, 512).transpose(1, 0, 2, 3).reshape(D, N)
        outs.append(a.T.astype(np.float32))
    return np.ascontiguousarray(np.stack(outs))


# revision 24
# speedup vs baseline: 1.2549x; 1.0271x over previous
"""Trainium2 Bass kernel for nn_Attention_84585085927925 — folded Gram chain.

Reference (per batch element b, fp32):
    qkv = x @ w_qkv.T ; q,k,v heads of 64 ; attn = sqrt(64) * q @ k.T (NO
    softmax) ; out = attn @ v ; out = out @ w_fc.T + b_fc

No softmax -> the map is linear and collapses into a per-batch weight:
    G_h  = wk_h (x^T x) wv_h^T                (64 x 64 per head)
    W    = sum_h (s*wq_h)^T G_h wfc_h^T       (768 x 768)
    out  = x @ W + b_fc
Per-core pipeline (one batch element per core, 8 cores):
    C    = x^T x        symmetric: 21 upper blocks (bf16) + 15 PE transposes
    M1T  = C @ wv^T     (f32r, = (wv C)^T)
    G'_t = diag 64-blocks of M1T_pair^T wk_pair  (bf16, narrow)
    GF   = blockdiag(G') @ wfc^T                 (f32r)
    W    = (s*wq)^T @ GF                         (f32r)
    outT = W^T-tiles @ x^T + bias                (f32r)
Measured rates: f32r matmul ~0.44 ns/col (>=256 wide), bf16 ~0.505 ns/col
(any width).  f32r for wide phases, bf16 for C (cheap x streaming, narrow
C blocks) and G'.  HBM tensors are host-prearranged to [128, ...] so every
DMA is 128 contiguous descriptors; DMA triggers cost ~700ns serialized on
their queue engine, so output triggers alternate sync/gpsimd.
"""

import numpy as np
import ml_dtypes

import concourse.bass as bass  # noqa: F401  (registers engine namespaces)
import concourse.mybir as mybir
import concourse.tile as tile
from concourse import bacc, bass_utils

F32 = mybir.dt.float32
F32R = mybir.dt.float32r
BF16 = mybir.dt.bfloat16

B, N, D, H = 8, 1024, 768, 12
HD = D // H            # 64
SCALE = float(np.sqrt(HD))
DT = D // 128          # 6 chunks of the feature axis
NT = N // 128          # 8 token tiles
NPAIR = H // 2         # 6 head pairs
ECH = 384              # column chunk (fits PSUM bank with headroom)


def _build_program():
    nc = bacc.Bacc(
        trn_type="TRN2", target_bir_lowering=False, debug=False, num_devices=B
    )
    xn_d = nc.dram_tensor("xn", [128, NT, D], BF16, kind="ExternalInput").ap()
    xt_d = nc.dram_tensor("xt", [128, DT, N], F32, kind="ExternalInput").ap()
    wvt_d = nc.dram_tensor("wvt", [128, DT, D], F32, kind="ExternalInput").ap()
    wkt_d = nc.dram_tensor("wkt", [128, DT, D], BF16,
                           kind="ExternalInput").ap()
    wq_d = nc.dram_tensor("wq", [128, DT, D], F32, kind="ExternalInput").ap()
    wfct_d = nc.dram_tensor("wfct", [128, DT, D], F32,
                            kind="ExternalInput").ap()
    idm_d = nc.dram_tensor("idm", [128, 128], BF16, kind="ExternalInput").ap()
    bias_d = nc.dram_tensor("bias", [128, DT], F32, kind="ExternalInput").ap()
    out_d = nc.dram_tensor("outT", [128, 12, 512], BF16,
                           kind="ExternalOutput").ap()

    with tile.TileContext(nc) as tc:
        with tc.tile_pool(name="big", bufs=1) as big, \
             tc.tile_pool(name="tmps", bufs=10) as tmps, \
             tc.tile_pool(name="outsp", bufs=4) as outsp, \
             tc.tile_pool(name="ps", bufs=7, space="PSUM") as ps:

            xn_sb = big.tile([128, NT, D], BF16, name="xn_sb")
            xt_sb = big.tile([128, DT, N], F32R, name="xt_sb")
            wvt_sb = big.tile([128, DT, D], F32R, name="wvt_sb")
            wkt_sb = big.tile([128, DT, D], BF16, name="wkt_sb")
            wq_sb = big.tile([128, DT, D], F32R, name="wq_sb")
            wfct_sb = big.tile([128, DT, D], F32R, name="wfct_sb")
            c_sb = big.tile([128, DT, D], F32R, name="c_sb")
            m1t_sb = big.tile([128, DT, D], BF16, name="m1t_sb")
            g2_sb = big.tile([128, NPAIR, 128], F32R, name="g2_sb")
            gf_sb = big.tile([128, DT, D], F32R, name="gf_sb")
            w_sb = big.tile([128, DT, D], F32R, name="w_sb")
            id_sb = big.tile([128, 128], BF16, name="id_sb")
            bias_sb = big.tile([128, DT], F32, name="bias_sb")
            warm_sb = big.tile([128, 512], BF16, name="warm_sb")

            xtr = xt_d.bitcast(F32R)
            wvtr = wvt_d.bitcast(F32R)
            wqr = wq_d.bitcast(F32R)
            wfctr = wfct_d.bitcast(F32R)

            # early memset off the critical engines
            nc.vector.memset(warm_sb[:], 0.0)

            # ---- DMA triggers: alternate sync/scalar so the ~700ns
            # per-trigger costs overlap; x first (C needs it) ----
            nc.sync.dma_start(xn_sb[:, 0:1, :], xn_d[:, 0:1, :])
            nc.scalar.dma_start(xn_sb[:, 1:2, :], xn_d[:, 1:2, :])
            nc.sync.dma_start(xn_sb[:, 2:4, :], xn_d[:, 2:4, :])
            nc.scalar.dma_start(xn_sb[:, 4:6, :], xn_d[:, 4:6, :])
            nc.sync.dma_start(xn_sb[:, 6:8, :], xn_d[:, 6:8, :])
            nc.scalar.dma_start(id_sb[:], idm_d[:])
            nc.sync.dma_start(wvt_sb[:], wvtr[:])
            nc.scalar.dma_start(wkt_sb[:], wkt_d[:])
            nc.sync.dma_start(wfct_sb[:], wfctr[:])
            nc.scalar.dma_start(wq_sb[:], wqr[:])
            nc.sync.dma_start(xt_sb[:], xtr[:])
            nc.scalar.dma_start(bias_sb[:], bias_d[:])

            # ---- PE warmup (p-state ramp) while x streams in ----
            wpt = ps.tile([128, 512], F32, tag="b", name="wpt")
            for i in range(3):
                nc.tensor.matmul(wpt[:], warm_sb[:, 0:128], warm_sb[:],
                                 start=(i == 0), stop=(i == 2))

            # ---- C = x^T x, upper triangle in two column passes (bf16).
            # Pass tile (d1, cols cs:ce) starts at the diagonal, so only
            # upper blocks are computed; lower blocks come from PE
            # transposes emitted in reverse-d order so M1T (d descending)
            # never waits.
            tmt = {}   # (d1, d2) -> bf16 staging tile for transpose

            def c_pass(ce_lo, ce_hi, rows):
                accs = {}
                for d1 in rows:
                    cs = max(ce_lo, d1 * 128)
                    acc = ps.tile([128, ce_hi - cs], F32, tag="b",
                                  name=f"cp{ce_lo}{d1}")
                    accs[d1] = (acc, cs)
                for nt in range(NT):
                    for d1 in rows:
                        acc, cs = accs[d1]
                        nc.tensor.matmul(
                            acc[:],
                            xn_sb[:, nt, d1 * 128:(d1 + 1) * 128],
                            xn_sb[:, nt, cs:ce_hi],
                            start=(nt == 0), stop=(nt == NT - 1),
                        )
                return accs

            def c_pass_copies(accs, ce_hi, copy_order):
                # copy_order: list of ('i', d1) full-tile or ('t', d1, d2)
                # transpose-staging entries, executed alternately on
                # vector/scalar
                for i, ent in enumerate(copy_order):
                    eng_v = (i % 2 == 0)
                    if ent[0] == 'i':
                        d1 = ent[1]
                        acc, cs = accs[d1]
                        dst = c_sb[:, d1, cs:ce_hi]
                        if eng_v:
                            nc.vector.tensor_copy(dst, acc[:])
                        else:
                            nc.scalar.copy(dst, acc[:])
                    else:
                        d1, d2 = ent[1], ent[2]
                        acc, cs = accs[d1]
                        tm = tmps.tile([128, 128], BF16, tag="tm",
                                       name=f"tm{d1}{d2}")
                        src = acc[:, d2 * 128 - cs:(d2 + 1) * 128 - cs]
                        if eng_v:
                            nc.vector.tensor_copy(tm[:], src)
                        else:
                            nc.scalar.copy(tm[:], src)
                        tmt[(d1, d2)] = tm

            def c_transpose(pairs):
                for d1, d2 in pairs:
                    pt = ps.tile([128, 128], BF16, tag="b", name=f"tp{d1}{d2}")
                    nc.tensor.transpose(pt[:], tmt.pop((d1, d2)), id_sb[:])
                    dst = c_sb[:, d2, d1 * 128:(d1 + 1) * 128]
                    if d2 % 2 == 0:
                        nc.scalar.copy(dst, pt[:])
                    else:
                        nc.vector.tensor_copy(dst, pt[:])

            accs_a = c_pass(0, ECH, rows=[0, 1, 2])
            c_pass_copies(accs_a, ECH,
                          [('i', 0), ('i', 1), ('i', 2),
                           ('t', 0, 1), ('t', 0, 2), ('t', 1, 2)])
            accs_b = c_pass(ECH, D, rows=[5, 4, 3, 2, 1, 0])
            c_transpose([(0, 1), (0, 2), (1, 2)])
            c_pass_copies(accs_b, D,
                          [('i', 5), ('i', 4), ('t', 4, 5),
                           ('i', 3), ('t', 3, 4), ('t', 3, 5),
                           ('i', 2), ('t', 2, 3), ('t', 2, 4), ('t', 2, 5),
                           ('i', 1), ('t', 1, 3), ('t', 1, 4), ('t', 1, 5),
                           ('i', 0), ('t', 0, 3), ('t', 0, 4), ('t', 0, 5)])

            # ---- M1T = C @ wv^T (f32r); d descending, transposes for
            # row d emitted just ahead of the chains that need them ----
            tr_sched = {5: [(4, 5)], 4: [(3, 4), (3, 5)],
                        3: [(2, 3), (2, 4), (2, 5)],
                        2: [(1, 3), (1, 4), (1, 5)],
                        1: [(0, 3), (0, 4), (0, 5)]}
            for d in range(DT - 1, -1, -1):
                c_transpose(tr_sched.get(d, []))
                for jc in range(2):
                    pt = ps.tile([128, ECH], F32, tag="b", name="pt_m1")
                    for k in range(DT - 1, -1, -1):
                        nc.tensor.matmul(
                            pt[:],
                            c_sb[:, k, d * 128:(d + 1) * 128],
                            wvt_sb[:, k, jc * ECH:(jc + 1) * ECH],
                            start=(k == DT - 1), stop=(k == 0),
                        )
                    dst = m1t_sb[:, d, jc * ECH:(jc + 1) * ECH]
                    if jc == 0:
                        nc.vector.tensor_copy(dst, pt[:])
                    else:
                        nc.scalar.copy(dst, pt[:])

            # ---- G' per head pair: diag 64-blocks of M1T_p^T @ wkT_p ----
            for t in range(NPAIR):
                gp = ps.tile([128, 128], F32, tag="b", name="gp")
                for k in range(DT):
                    nc.tensor.matmul(
                        gp[:],
                        m1t_sb[:, k, t * 128:(t + 1) * 128],
                        wkt_sb[:, k, t * 128:(t + 1) * 128],
                        start=(k == 0), stop=(k == DT - 1),
                    )
                nc.vector.tensor_scalar_mul(g2_sb[:, t, :], gp[:], 0.0)
                nc.vector.tensor_copy(g2_sb[0:64, t, 0:64], gp[0:64, 0:64])
                nc.vector.tensor_copy(g2_sb[64:128, t, 64:128],
                                      gp[64:128, 64:128])

            # ---- GF = blockdiag(G') @ wfc^T (f32r) ----
            for jc in range(2):
                for t in range(NPAIR):
                    pt = ps.tile([128, ECH], F32, tag="b", name="pt_gf")
                    nc.tensor.matmul(
                        pt[:],
                        g2_sb[:, t, :],
                        wfct_sb[:, t, jc * ECH:(jc + 1) * ECH],
                        start=True, stop=True,
                    )
                    dst = gf_sb[:, t, jc * ECH:(jc + 1) * ECH]
                    if t % 2 == 0:
                        nc.vector.tensor_copy(dst, pt[:])
                    else:
                        nc.scalar.copy(dst, pt[:])

            # ---- W = (s*wq)^T @ GF (f32r) ----
            for jc in range(2):
                for d in range(DT):
                    pt = ps.tile([128, ECH], F32, tag="b", name="pt_w")
                    for k in range(DT):
                        nc.tensor.matmul(
                            pt[:],
                            wq_sb[:, k, d * 128:(d + 1) * 128],
                            gf_sb[:, k, jc * ECH:(jc + 1) * ECH],
                            start=(k == 0), stop=(k == DT - 1),
                        )
                    dst = w_sb[:, d, jc * ECH:(jc + 1) * ECH]
                    if d % 2 == 0:
                        nc.vector.tensor_copy(dst, pt[:])
                    else:
                        nc.scalar.copy(dst, pt[:])

            # ---- outT[j, n] = sum_d W[d, j] x^T[d, n] + bias ----
            for jt in range(DT):
                for ic in range(2):
                    pt = ps.tile([128, 512], F32, tag="b", name="pt_o")
                    for k in range(DT):
                        nc.tensor.matmul(
                            pt[:],
                            w_sb[:, k, jt * 128:(jt + 1) * 128],
                            xt_sb[:, k, ic * 512:(ic + 1) * 512],
                            start=(k == 0), stop=(k == DT - 1),
                        )
                    ot = outsp.tile([128, 512], BF16, tag="ot", name="ot")
                    last = (jt == DT - 1 and ic == 1)
                    if not last:
                        # add + trigger both on scalar: no cross-engine hop
                        nc.scalar.add(ot[:], pt[:], bias_sb[:, jt:jt + 1])
                        nc.scalar.dma_start(out_d[:, jt * 2 + ic, :], ot[:])
                    else:
                        # split the tail: scalar and vector halves in
                        # parallel, DMA triggers on scalar and sync
                        nc.vector.tensor_scalar_add(
                            ot[:, 256:512], pt[:, 256:512],
                            bias_sb[:, jt:jt + 1])
                        nc.scalar.add(ot[:, 0:256], pt[:, 0:256],
                                      bias_sb[:, jt:jt + 1])
                        nc.scalar.dma_start(out_d[:, jt * 2 + ic, 0:256],
                                            ot[:, 0:256])
                        nc.sync.dma_start(out_d[:, jt * 2 + ic, 256:512],
                                          ot[:, 256:512])

    nc.compile()
    return nc


_NC_CACHE = None
LAST_EXEC_NS = None
LAST_RES = None


def _arr128(a):
    """[D0*128, M] row-major -> [128, D0, M] partition-major, contiguous."""
    d0 = a.shape[0] // 128
    return np.ascontiguousarray(
        a.reshape(d0, 128, a.shape[1]).transpose(1, 0, 2))


def kernel(x, w_qkv, w_fc, b_fc, _trace=False):
    global _NC_CACHE, LAST_EXEC_NS, LAST_RES
    x = np.asarray(x, dtype=np.float32)
    w_qkv = np.asarray(w_qkv, dtype=np.float32)
    w_fc = np.asarray(w_fc, dtype=np.float32)
    b_fc = np.asarray(b_fc, dtype=np.float32)

    if _NC_CACHE is None:
        _NC_CACHE = _build_program()
    nc = _NC_CACHE

    bf = ml_dtypes.bfloat16
    wq = _arr128(SCALE * w_qkv[:D])                       # [128, 6, 768] f32
    wkt = _arr128(np.ascontiguousarray(w_qkv[D:2 * D].T).astype(bf))
    wvt = _arr128(np.ascontiguousarray(w_qkv[2 * D:].T))
    wfct = _arr128(np.ascontiguousarray(w_fc.T))
    bias = np.ascontiguousarray(b_fc.reshape(DT, 128).T)  # [128, 6] f32
    idm = np.eye(128, dtype=bf)

    in_maps = []
    for b in range(B):
        in_maps.append({
            "xn": _arr128(x[b].astype(bf)),               # [128, 8, 768]
            "xt": _arr128(np.ascontiguousarray(x[b].T)),  # [128, 6, 1024] f32
            "wvt": wvt, "wkt": wkt, "wq": wq, "wfct": wfct,
            "idm": idm, "bias": bias,
        })

    res = bass_utils.run_bass_kernel_spmd(
        nc, in_maps, core_ids=list(range(B)), trace=_trace
    )
    LAST_EXEC_NS = res.exec_time_ns
    LAST_RES = res
    outs = []
    for b in range(B):
        a = res.results[b]["outT"]                        # [128, 12, 512] bf16
        a = a.reshape(128, DT, 2, 512).transpose(1, 0, 2, 3).reshape(D, N)
        outs.append(a.T.astype(np.float32))
    return np.ascontiguousarray(np.stack(outs))
